# revision 8
# baseline (speedup 1.0000x reference)
"""GATv2 message-passing (4 layers) on 8 Trainium2 NeuronCores.

Self-contained kernel: takes the FULL inputs of the reference problem
(x [100000,2], edge_index [2,1000000] int64, edge_attr [1000000,2],
batch [100000] int64, mask [100000], params pytree) and returns the full
(x_out [100000], x_out_sat [64]) outputs.

Strategy
--------
- Nodes are split into 8 contiguous shards (12500/core, padded to 98*128
  rows).  Edges are owned by the core that owns their dst node and sorted
  by dst.  Each core computes the GAT aggregation for its own dst nodes.
- Per layer, each node needs xl[src] for arbitrary src -> the projected
  xl tables are AllGather'd across the 8 cores; xr stays shard-local.
- Edges are processed in 128-edge chunks grouped under 128-node dst
  tiles.  Chunk counts per tile are padded to the max over cores so one
  SPMD program fits all cores (dead edges carry dst_local=-1 and fall out
  of the one-hot scatter).
- Per chunk: indirect-DMA gather of xl[src] rows; TensorE matmuls expand
  xr[dst] (one-hot), project edge_attr, and scatter exp(logit)*xl /
  exp(logit) sums into a PSUM accumulator per node tile; DVE/ACT compute
  leaky_relu, per-head logits and exp.
- The softmax is computed unnormalized (out_i = sum_e w_e*xl_src / sum_e
  w_e with w = exp(logit)); logits are O(1) so this is numerically safe.
"""

import os
import sys

sys.path.insert(0, "/opt/trn_rl_repo")

import numpy as np

import concourse.bass as bass
import concourse.mybir as mybir
import concourse.tile as tile
import concourse.bacc as bacc
from concourse.masks import make_identity

P = 128
F32 = mybir.dt.float32
I32 = mybir.dt.int32
I16 = mybir.dt.int16

NEG_ATT = 0.2
NEG_ACT = 0.01


class Cfg:
    def __init__(self, n=100_000, e=1_000_000, b=64, heads=16, ncores=8, g=32):
        self.N = n
        self.E = e
        self.B = b
        self.H = heads
        self.NC = ncores
        self.G = g                      # chunks per superchunk (DMA batch)
        self.NSH = n // ncores          # real nodes per core
        self.NT = (self.NSH + P - 1) // P   # node tiles per core
        self.ROWS = self.NT * P         # padded rows per core shard


# ----------------------------------------------------------------------------
# Host-side preprocessing: edge sharding, sorting, chunk/tile layout
# ----------------------------------------------------------------------------

def prep_host(x, edge_index, edge_attr, batch, mask, cfg: Cfg):
    N, NC, NSH, NT, G = cfg.N, cfg.NC, cfg.NSH, cfg.NT, cfg.G
    src = np.asarray(edge_index[0], dtype=np.int64)
    dst = np.asarray(edge_index[1], dtype=np.int64)
    ea = np.asarray(edge_attr, dtype=np.float32)

    core_of = dst // NSH                       # owning core per edge
    core_of = np.minimum(core_of, NC - 1)      # (N % NC == 0 here, no-op)

    # per-core edge lists sorted by dst
    per_core = []
    for c in range(NC):
        sel = np.nonzero(core_of == c)[0]
        order = np.argsort(dst[sel], kind="stable")
        eid = sel[order]
        per_core.append((src[eid], dst[eid], ea[eid]))

    # edges per (core, tile); tile = (dst - c*NSH) // P
    cnt = np.zeros((NC, NT), dtype=np.int64)
    for c in range(NC):
        tloc = (per_core[c][1] - c * NSH) // P
        cnt[c] = np.bincount(tloc, minlength=NT)
    ch = np.maximum(np.ceil(cnt / P).astype(np.int64).max(axis=0), 1)  # [NT]
    totch = int(ch.sum())
    totch_pad = ((totch + G - 1) // G) * G
    ch[NT - 1] += totch_pad - totch
    totch = totch_pad
    tot_e = totch * P

    # tile id / first / last flags per chunk (shared across cores)
    tile_of_chunk = np.repeat(np.arange(NT), ch)
    first = np.zeros(totch, dtype=bool)
    last = np.zeros(totch, dtype=bool)
    off = 0
    for t in range(NT):
        first[off] = True
        off += int(ch[t])
        last[off - 1] = True

    in_maps = []
    x_np = np.asarray(x, dtype=np.float32)
    batch_np = np.asarray(batch, dtype=np.int64)
    mask_np = np.asarray(mask, dtype=np.float32)
    for c in range(NC):
        s, d, a = per_core[c]
        # fill per-tile padded edge arrays (chunk-major flat order)
        gidxr = np.zeros(tot_e, dtype=np.int32)      # remapped src row
        dl = np.full(tot_e, -1.0, dtype=np.float32)  # dst_local or -1
        eaT = np.zeros((2, tot_e), dtype=np.float32)
        xeaT = np.zeros((4, tot_e), dtype=np.float32)  # [x0;x1;a0;a1] per edge
        tloc = (d - c * NSH) // P
        bnd = np.searchsorted(tloc, np.arange(NT + 1))
        off = 0
        for t in range(NT):
            lo, hi = bnd[t], bnd[t + 1]
            n = hi - lo
            gidxr[off : off + n] = (s[lo:hi] // NSH) * cfg.ROWS + (s[lo:hi] % NSH)
            dl[off : off + n] = (d[lo:hi] - c * NSH - t * P).astype(np.float32)
            eaT[:, off : off + n] = a[lo:hi].T
            xeaT[0:2, off : off + n] = x_np[s[lo:hi]].T
            xeaT[2:4, off : off + n] = a[lo:hi].T
            off += int(ch[t]) * P
        # chunk-major [nchunks,128] -> transposed [128, nchunks] layouts
        gidxr_t = gidxr.reshape(totch, P).T.copy()
        dlcolT = dl.reshape(totch, P).T.copy()
        dlrow = dl.astype(np.int16)[None, :]

        # shard-local node data (padded to ROWS)
        xsh = np.zeros((cfg.ROWS, 2), dtype=np.float32)
        xsh[:NSH] = x_np[c * NSH : (c + 1) * NSH]
        bcol = np.full(cfg.ROWS, -1.0, dtype=np.float32)
        bcol[:NSH] = batch_np[c * NSH : (c + 1) * NSH].astype(np.float32)
        mcol = np.zeros(cfg.ROWS, dtype=np.float32)
        mcol[:NSH] = mask_np[c * NSH : (c + 1) * NSH]

        in_maps.append(
            {
                "xshT": xsh.T.copy(),
                "gidxr": gidxr_t,
                "dlrow": dlrow,
                "dlcolT": dlcolT,
                "eattrT": eaT,
                "xeaT": xeaT,
                "batch_colT": bcol.reshape(NT, P).T.copy(),
                "mask_colT": mcol.reshape(NT, P).T.copy(),
            }
        )

    meta = {
        "ch": ch,
        "totch": totch,
        "tile_of_chunk": tile_of_chunk,
        "first": first,
        "last": last,
    }
    return in_maps, meta


def pack_params(params, cfg: Cfg):
    """Parameter tensors shared by all cores (replicated)."""
    p0, p1, pf, ps = params["l0"], params["l1"], params["final"], params["sbs"]
    f = lambda v: np.asarray(v, dtype=np.float32)
    out = {
        "W0r": f(p0["Wr"]),                            # [2,128]
        "WE0": f(p0["We"]),                            # [2,128]
        "W1cat": np.concatenate([f(p1["Wl"]), f(p1["Wr"])], axis=1),   # [128,256]
        "WE1": f(p1["We"]),                            # [2,128]
        "W0l": f(p0["Wl"]),                            # [2,128]
        "W0lWE0": np.concatenate([f(p0["Wl"]), f(p0["We"])], axis=0),  # [4,128]
        "Wfscat": np.concatenate(
            [f(pf["Wl"]), f(pf["Wr"]), f(ps["Wl"]), f(ps["Wr"])], axis=1
        ),                                             # [128,64]
        "WEfs": np.concatenate([f(pf["We"]), f(ps["We"])], axis=1),    # [2,32]
        "att0_row": f(p0["att"]).reshape(1, -1),       # [1,128]
        "att1_row": f(p1["att"]).reshape(1, -1),       # [1,128]
        "attfs_row": np.concatenate(
            [f(pf["att"]).reshape(-1), f(ps["att"]).reshape(-1)]
        ).reshape(1, -1),                              # [1,32]
        "b0sum_row": (f(p0["bl"]) + f(p0["br"])).reshape(1, -1),       # [1,128]
        "b1sum_row": (f(p1["bl"]) + f(p1["br"])).reshape(1, -1),       # [1,128]
        "bfssum_row": np.concatenate(
            [f(pf["bl"]) + f(pf["br"]), f(ps["bl"]) + f(ps["br"])]
        ).reshape(1, -1),                              # [1,32]
        "hbias0_row": f(p0["bias"]).reshape(1, -1),    # [1,128]
        "hbias1_row": f(p1["bias"]).reshape(1, -1),    # [1,128]
        "bl0_row": f(p0["bl"]).reshape(1, -1),         # [1,128]
        "bl1_row": f(p1["bl"]).reshape(1, -1),         # [1,128]
        "blfs_row": np.concatenate(
            [f(pf["bl"]), f(ps["bl"])]
        ).reshape(1, -1),                              # [1,32]
    }
    scalars = {
        "bias_f": float(np.asarray(pf["bias"]).reshape(-1)[0]),
        "bias_s": float(np.asarray(ps["bias"]).reshape(-1)[0]),
    }
    return out, scalars


# ----------------------------------------------------------------------------
# Device kernel builder
# ----------------------------------------------------------------------------

def build_kernel(meta, pshapes, scalars, cfg: Cfg):
    N, NT, G, H, NC = cfg.N, cfg.NT, cfg.G, cfg.H, cfg.NC
    ROWS = cfg.ROWS
    totch = meta["totch"]
    tile_of_chunk = meta["tile_of_chunk"]
    first, last = meta["first"], meta["last"]
    C = 128 // H  # per-head channels of the big layers (8)

    nc = bacc.Bacc(
        "TRN2", target_bir_lowering=False, debug=False, num_devices=NC
    )

    din = {}
    def d_in(name, shape, dt=F32):
        din[name] = nc.dram_tensor(name, list(shape), dt, kind="ExternalInput")
        return din[name]

    t_xshT = d_in("xshT", (2, ROWS))
    t_gidxr = d_in("gidxr", (P, totch), I32)
    t_dlrow = d_in("dlrow", (1, totch * P), I16)
    t_dlcolT = d_in("dlcolT", (P, totch))
    t_eattrT = d_in("eattrT", (2, totch * P))
    t_xeaT = d_in("xeaT", (4, totch * P))
    t_bcolT = d_in("batch_colT", (P, NT))
    t_mcolT = d_in("mask_colT", (P, NT))
    for name, shape in pshapes.items():
        d_in(name, shape)

    t_xout = nc.dram_tensor("x_out_sh", [P, NT], F32, kind="ExternalOutput")
    t_gsum = nc.dram_tensor("gsum", [cfg.B, 1], F32, kind="ExternalOutput")

    # internal DRAM
    t_xl1_sh = nc.dram_tensor("xl1_sh", [ROWS, 128], F32)
    t_xl1_full = nc.dram_tensor("xl1_full", [ROWS * NC, 128], F32)
    t_xlfs_sh = nc.dram_tensor("xlfs_sh", [ROWS, 32], F32)
    t_xlfs_full = nc.dram_tensor("xlfs_full", [ROWS * NC, 32], F32)
    t_xr1 = nc.dram_tensor("xr1_dram", [ROWS, 128], F32)
    t_dbg_h1 = nc.dram_tensor("dbg_h1", [ROWS, 128], F32) if os.environ.get("K_DEBUG") else None
    t_dbg_out0 = nc.dram_tensor("dbg_out0", [ROWS, 128], F32) if os.environ.get("K_DEBUG") else None
    t_dbg_den0 = nc.dram_tensor("dbg_den0", [ROWS, 16], F32) if os.environ.get("K_DEBUG") else None
    t_xrfs = nc.dram_tensor("xrfs_dram", [ROWS, 32], F32)

    groups = [list(range(NC))]

    with tile.TileContext(nc) as tc:
        with (
            tc.tile_pool(name="const", bufs=1) as cp,
            tc.tile_pool(name="sc", bufs=2) as sc,      # superchunk streams
            tc.tile_pool(name="wk", bufs=3) as wk,      # per-chunk work tiles
            tc.tile_pool(name="nd", bufs=2) as ndp,     # node-phase tiles
            tc.tile_pool(name="pre_ps", bufs=2, space="PSUM") as pre_ps,
            tc.tile_pool(name="scat_ps", bufs=2, space="PSUM") as scat_ps,
            tc.tile_pool(name="aux_ps", bufs=2, space="PSUM") as aux_ps,
            tc.tile_pool(name="proj_ps", bufs=2, space="PSUM") as proj_ps,
        ):
            # ---------------- constants
            ident = cp.tile([P, P], F32)
            make_identity(nc, ident[:])
            iota_row_i = cp.tile([P, P], I32)     # [p,f] = f
            nc.gpsimd.iota(iota_row_i[:], pattern=[[1, P]], base=0, channel_multiplier=0)
            iota_row = cp.tile([P, P], F32)
            nc.vector.tensor_copy(iota_row[:], iota_row_i[:])
            iota64_row_i = cp.tile([P, cfg.B], I32)
            nc.gpsimd.iota(iota64_row_i[:], pattern=[[1, cfg.B]], base=0, channel_multiplier=0)
            iota64_row = cp.tile([P, cfg.B], F32)
            nc.vector.tensor_copy(iota64_row[:], iota64_row_i[:])
            iota_col_i = cp.tile([P, 1], I32)
            nc.gpsimd.iota(iota_col_i[:], pattern=[[0, 1]], base=0, channel_multiplier=1)
            iota_col = cp.tile([P, 1], F32)
            nc.vector.tensor_copy(iota_col[:], iota_col_i[:])

            def bcast_const(name, width):
                t = cp.tile([P, width], F32, tag=f"c_{name}")
                nc.sync.dma_start(t[:], din[name][:, :].partition_broadcast(P))
                return t

            att0_b = bcast_const("att0_row", 128)
            att1_b = bcast_const("att1_row", 128)
            attfs_b = bcast_const("attfs_row", 32)
            b0sum_b = bcast_const("b0sum_row", 128)
            b1sum_b = bcast_const("b1sum_row", 128)
            bfssum_b = bcast_const("bfssum_row", 32)
            hbias0_b = bcast_const("hbias0_row", 128)
            hbias1_b = bcast_const("hbias1_row", 128)
            bl0_b = bcast_const("bl0_row", 128)
            bl1_b = bcast_const("bl1_row", 128)
            blfs_b = bcast_const("blfs_row", 32)

            def load_w(name, shape):
                t = cp.tile(list(shape), F32, tag=f"w_{name}")
                nc.sync.dma_start(t[:], din[name][:, :])
                return t

            W0r = load_w("W0r", (2, 128))
            W0l = load_w("W0l", (2, 128))
            W0lWE0 = load_w("W0lWE0", (4, 128))
            W1cat = load_w("W1cat", (128, 256))
            WE1 = load_w("WE1", (2, 128))
            Wfscat = load_w("Wfscat", (128, 64))
            WEfs = load_w("WEfs", (2, 32))
            xshT_sb = load_w("xshT", (2, ROWS))

            # output accumulators
            xout_sb = cp.tile([P, NT], F32)
            gsum_ps = aux_ps.tile([cfg.B, 1], F32, space="PSUM", tag="gsum", bufs=1)

            # ------------------------------------------------------------------
            # generic edge-phase + node-phase pass over one GAT layer
            # ------------------------------------------------------------------
            def gat_layer(layer):
                """layer in {0, 1, 2}; 2 = fused final+sbs."""
                if layer == 0:
                    FW = 0        # no device gather: x[src] comes as a host stream
                    gidx, table = None, None
                elif layer == 1:
                    FW = 128
                    gidx, table = t_gidxr, t_xl1_full
                else:
                    FW = 32
                    gidx, table = t_gidxr, t_xlfs_full
                DW = 128 if layer < 2 else 32    # working feature width
                HW = H if layer < 2 else 2 * H   # logits width
                CH_ = DW // HW                   # per-head channels (8 or 1)

                scat = None
                xr_t = None
                xsT_sb = None

                for j in range(totch):
                    g = j % G
                    if g == 0:
                        # ---------- superchunk DMA batch
                        if layer > 0:
                            idx_sb = sc.tile([P, G], I32, tag="idx")
                            nc.sync.dma_start(idx_sb[:], gidx[:, j : j + G])
                        dlb_sb = sc.tile([P, G * P], I16, tag="dlb")
                        nc.sync.dma_start(
                            dlb_sb[:],
                            t_dlrow[:, j * P : (j + G) * P].partition_broadcast(P),
                        )
                        dlc_sb = sc.tile([P, G], F32, tag="dlc")
                        nc.sync.dma_start(dlc_sb[:], t_dlcolT[:, j : j + G])
                        if layer == 0:
                            ea_sb = sc.tile([4, G * P], F32, tag="xea")
                            nc.sync.dma_start(ea_sb[:], t_xeaT[:, j * P : (j + G) * P])
                        else:
                            ea_sb = sc.tile([2, G * P], F32, tag="ea")
                            nc.sync.dma_start(ea_sb[:], t_eattrT[:, j * P : (j + G) * P])

                    if layer > 0:
                        gat_sb = wk.tile([P, FW], F32, tag="gat")
                        nc.gpsimd.indirect_dma_start(
                            out=gat_sb[:],
                            out_offset=None,
                            in_=table[:, :],
                            in_offset=bass.IndirectOffsetOnAxis(ap=idx_sb[:, g : g + 1], axis=0),
                        )

                    t = int(tile_of_chunk[j])
                    if first[j]:
                        # ---------- per-tile setup: xr tile + fresh scatter psum
                        scat = scat_ps.tile([P, DW + HW], F32, space="PSUM", tag="scat")
                        if layer == 0:
                            xr_ps = pre_ps.tile([P, 128], F32, space="PSUM", tag="pre")
                            nc.tensor.matmul(
                                xr_ps[:],
                                lhsT=xshT_sb[:, t * P : (t + 1) * P],
                                rhs=W0r[:],
                                start=True, stop=True,
                            )
                            xr_t = wk.tile([P, 128], F32, tag="xrt")
                            nc.vector.tensor_add(xr_t[:], xr_ps[:], b0sum_b[:])
                        else:
                            xr_t = wk.tile([P, DW], F32, tag="xrt")
                            src_dram = t_xr1 if layer == 1 else t_xrfs
                            nc.sync.dma_start(
                                xr_t[:], src_dram[t * P : (t + 1) * P, :]
                            )

                    # ---------- one-hot builds
                    oh_sc = wk.tile([P, P], F32, tag="ohsc")
                    nc.vector.tensor_scalar(
                        out=oh_sc[:], in0=iota_row[:],
                        scalar1=dlc_sb[:, g : g + 1], scalar2=None,
                        op0=mybir.AluOpType.is_equal,
                    )
                    oh_ex = wk.tile([P, P], F32, tag="ohex")
                    nc.gpsimd.tensor_scalar(
                        out=oh_ex[:], in0=dlb_sb[:, g * P : (g + 1) * P],
                        scalar1=iota_col[:, :1], scalar2=None,
                        op0=mybir.AluOpType.is_equal,
                    )

                    # ---------- pre-activation matmuls
                    pre = pre_ps.tile([P, DW], F32, space="PSUM", tag="pre")
                    nc.tensor.matmul(pre[:], lhsT=oh_ex[:], rhs=xr_t[:], start=True, stop=False)
                    if layer == 0:
                        # xl0_src + eproj accumulated into pre; xl0 kept separately
                        nc.tensor.matmul(
                            pre[:], lhsT=ea_sb[:, g * P : (g + 1) * P], rhs=W0lWE0[:],
                            start=False, stop=True,
                        )
                        xl0 = proj_ps.tile([P, 128], F32, space="PSUM", tag="xl0")
                        nc.tensor.matmul(
                            xl0[:], lhsT=ea_sb[0:2, g * P : (g + 1) * P], rhs=W0l[:],
                            start=True, stop=True,
                        )
                        m = pre
                        xl_src = xl0
                    else:
                        WEx = WE1 if layer == 1 else WEfs
                        nc.tensor.matmul(
                            pre[:], lhsT=ea_sb[:, g * P : (g + 1) * P], rhs=WEx[:],
                            start=False, stop=True,
                        )
                        m = wk.tile([P, DW], F32, tag="m")
                        nc.vector.tensor_add(m[:], gat_sb[:], pre[:])
                        xl_src = gat_sb[:]

                    # ---------- leaky_relu(0.2) + per-head logits
                    m02 = wk.tile([P, DW], F32, tag="m02")
                    nc.scalar.activation(
                        m02[:], m[:], mybir.ActivationFunctionType.Copy, bias=0.0, scale=NEG_ATT
                    )
                    mr = wk.tile([P, DW], F32, tag="mr")
                    nc.vector.tensor_tensor(out=mr[:], in0=m[:], in1=m02[:], op=mybir.AluOpType.max)
                    attb = att0_b if layer == 0 else (att1_b if layer == 1 else attfs_b)
                    lg = wk.tile([P, DW], F32, tag="lg")
                    nc.vector.tensor_tensor(out=lg[:], in0=mr[:], in1=attb[:, :DW], op=mybir.AluOpType.mult)
                    # rhs for scatter matmul: [values | w]
                    rhs = wk.tile([P, DW + HW], F32, tag="rhs")
                    if CH_ > 1:
                        logits = wk.tile([P, HW], F32, tag="logit")
                        nc.vector.tensor_reduce(
                            out=logits[:],
                            in_=lg[:].rearrange("p (h c) -> p h c", h=HW),
                            axis=mybir.AxisListType.X, op=mybir.AluOpType.add,
                        )
                    else:
                        logits = lg
                    nc.scalar.activation(
                        rhs[:, DW : DW + HW], logits[:],
                        mybir.ActivationFunctionType.Exp, bias=0.0, scale=1.0,
                    )
                    if CH_ > 1:
                        wb = rhs[:, DW : DW + HW][:, :, None].to_broadcast([P, HW, CH_])
                        nc.vector.tensor_tensor(
                            out=rhs[:, 0:DW].rearrange("p (h c) -> p h c", h=HW),
                            in0=xl_src[:].rearrange("p (h c) -> p h c", h=HW),
                            in1=wb, op=mybir.AluOpType.mult,
                        )
                    else:
                        nc.vector.tensor_tensor(
                            out=rhs[:, 0:DW], in0=xl_src[:], in1=rhs[:, DW : DW + HW],
                            op=mybir.AluOpType.mult,
                        )

                    # ---------- scatter accumulate
                    nc.tensor.matmul(
                        scat[:], lhsT=oh_sc[:], rhs=rhs[:],
                        start=first[j], stop=last[j],
                    )

                    if last[j]:
                        node_phase(layer, t, scat)

            # ------------------------------------------------------------------
            def node_phase(layer, t, scat):
                DW = 128 if layer < 2 else 32
                HW = H if layer < 2 else 2 * H
                CH_ = DW // HW
                # guarded reciprocal of denominators + (denom>0) gate for bl
                den = ndp.tile([P, HW], F32, tag="den")
                nc.vector.tensor_scalar(
                    out=den[:], in0=scat[:, DW : DW + HW],
                    scalar1=1e-30, scalar2=None, op0=mybir.AluOpType.max,
                )
                gate = ndp.tile([P, HW], F32, tag="gate")
                nc.vector.tensor_scalar(
                    out=gate[:], in0=scat[:, DW : DW + HW],
                    scalar1=0.0, scalar2=None, op0=mybir.AluOpType.is_gt,
                )
                rec = ndp.tile([P, HW], F32, tag="rec")
                nc.vector.reciprocal(rec[:], den[:])
                out_t = ndp.tile([P, DW], F32, tag="outt")
                if CH_ > 1:
                    nc.vector.tensor_tensor(
                        out=out_t[:].rearrange("p (h c) -> p h c", h=HW),
                        in0=scat[:, 0:DW].rearrange("p (h c) -> p h c", h=HW),
                        in1=rec[:, :, None].to_broadcast([P, HW, CH_]),
                        op=mybir.AluOpType.mult,
                    )
                else:
                    nc.vector.tensor_tensor(
                        out=out_t[:], in0=scat[:, 0:DW], in1=rec[:], op=mybir.AluOpType.mult
                    )

                if layer < 2:
                    blb = bl0_b if layer == 0 else bl1_b
                    blg = ndp.tile([P, DW], F32, tag="blg")
                    nc.vector.tensor_tensor(
                        out=blg[:].rearrange("p (h c) -> p h c", h=HW),
                        in0=blb[:].rearrange("p (h c) -> p h c", h=HW),
                        in1=gate[:, :, None].to_broadcast([P, HW, CH_]),
                        op=mybir.AluOpType.mult,
                    )
                    nc.vector.tensor_add(out_t[:], out_t[:], blg[:])
                    hb = hbias0_b if layer == 0 else hbias1_b
                    hb_t = ndp.tile([P, DW], F32, tag="hbt")
                    nc.vector.tensor_add(hb_t[:], out_t[:], hb[:])
                    h02 = ndp.tile([P, DW], F32, tag="h02")
                    nc.scalar.activation(
                        h02[:], hb_t[:], mybir.ActivationFunctionType.Copy,
                        bias=0.0, scale=NEG_ACT,
                    )
                    h_t = ndp.tile([P, DW], F32, tag="ht")
                    nc.vector.tensor_tensor(out=h_t[:], in0=hb_t[:], in1=h02[:], op=mybir.AluOpType.max)
                    if layer == 0 and t_dbg_h1 is not None:
                        nc.sync.dma_start(t_dbg_h1[t * P : (t + 1) * P, :], h_t[:])
                        nc.sync.dma_start(t_dbg_out0[t * P : (t + 1) * P, :], out_t[:])
                        nc.sync.dma_start(t_dbg_den0[t * P : (t + 1) * P, :], den[:])
                    # transpose h for the next-layer projections
                    hT_ps = pre_ps.tile([P, P], F32, space="PSUM", tag="pre")
                    nc.tensor.transpose(out=hT_ps[:], in_=h_t[:], identity=ident[:])
                    hT_sb = ndp.tile([P, P], F32, tag="hTsb")
                    nc.vector.tensor_copy(hT_sb[:], hT_ps[:])
                    Wn = W1cat if layer == 0 else Wfscat
                    PW = 256 if layer == 0 else 64
                    pj = proj_ps.tile([P, PW], F32, space="PSUM", tag="proj", bufs=1)
                    nc.tensor.matmul(pj[:], lhsT=hT_sb[:], rhs=Wn[:], start=True, stop=True)
                    XLW = 128 if layer == 0 else 32
                    xl_sb = ndp.tile([P, XLW], F32, tag="xlsb")
                    if layer == 0:
                        nc.vector.tensor_copy(xl_sb[:], pj[:, 0:128])
                        xr_sb = ndp.tile([P, 128], F32, tag="xrnsb")
                        nc.vector.tensor_add(xr_sb[:], pj[:, 128:256], b1sum_b[:])
                        nc.sync.dma_start(t_xl1_sh[t * P : (t + 1) * P, :], xl_sb[:])
                        nc.sync.dma_start(t_xr1[t * P : (t + 1) * P, :], xr_sb[:])
                    else:
                        # pj = [xlf | xrf | xls | xrs] each 16 wide
                        nc.vector.tensor_copy(xl_sb[:, 0:16], pj[:, 0:16])
                        nc.vector.tensor_copy(xl_sb[:, 16:32], pj[:, 32:48])
                        xr_sb = ndp.tile([P, 32], F32, tag="xrnsb2")
                        nc.vector.tensor_copy(xr_sb[:, 0:16], pj[:, 16:32])
                        nc.vector.tensor_copy(xr_sb[:, 16:32], pj[:, 48:64])
                        nc.vector.tensor_add(xr_sb[:], xr_sb[:], bfssum_b[:])
                        nc.sync.dma_start(t_xlfs_sh[t * P : (t + 1) * P, :], xl_sb[:])
                        nc.sync.dma_start(t_xrfs[t * P : (t + 1) * P, :], xr_sb[:])
                else:
                    # out_t = [final(16) | sbs(16)] per-head outputs
                    blg = ndp.tile([P, DW], F32, tag="blg2")
                    nc.vector.tensor_tensor(
                        out=blg[:], in0=blfs_b[:], in1=gate[:], op=mybir.AluOpType.mult
                    )
                    nc.vector.tensor_add(out_t[:], out_t[:], blg[:])
                    rs = ndp.tile([P, 2], F32, tag="rs")
                    nc.vector.tensor_reduce(
                        out=rs[:], in_=out_t[:].rearrange("p (s h) -> p s h", s=2),
                        axis=mybir.AxisListType.X, op=mybir.AluOpType.add,
                    )
                    xo = ndp.tile([P, 1], F32, tag="xo")
                    nc.vector.tensor_scalar(
                        out=xo[:], in0=rs[:, 0:1], scalar1=1.0 / H,
                        scalar2=scalars["bias_f"], op0=mybir.AluOpType.mult,
                        op1=mybir.AluOpType.add,
                    )
                    mcol = ndp.tile([P, 1], F32, tag="mcol")
                    nc.sync.dma_start(mcol[:], t_mcolT[:, t : t + 1])
                    nc.vector.tensor_tensor(
                        out=xout_sb[:, t : t + 1], in0=xo[:], in1=mcol[:], op=mybir.AluOpType.mult
                    )
                    xb = ndp.tile([P, 1], F32, tag="xb")
                    nc.vector.tensor_scalar(
                        out=xb[:], in0=rs[:, 1:2], scalar1=1.0 / H,
                        scalar2=scalars["bias_s"], op0=mybir.AluOpType.mult,
                        op1=mybir.AluOpType.add,
                    )
                    bcol = ndp.tile([P, 1], F32, tag="bcol")
                    nc.sync.dma_start(bcol[:], t_bcolT[:, t : t + 1])
                    oh_b = ndp.tile([P, cfg.B], F32, tag="ohb")
                    nc.vector.tensor_scalar(
                        out=oh_b[:], in0=iota64_row[:], scalar1=bcol[:, :1],
                        scalar2=None, op0=mybir.AluOpType.is_equal,
                    )
                    nc.tensor.matmul(
                        gsum_ps[:], lhsT=oh_b[:], rhs=xb[:],
                        start=(t == 0), stop=(t == NT - 1),
                    )
                    if t == NT - 1:
                        gs_sb = ndp.tile([cfg.B, 1], F32, tag="gssb")
                        nc.vector.tensor_copy(gs_sb[:], gsum_ps[:])
                        nc.sync.dma_start(t_gsum[:, :], gs_sb[:])

            # ---------------- run the layers
            gat_layer(0)
            nc.gpsimd.collective_compute(
                "AllGather", mybir.AluOpType.bypass, replica_groups=groups,
                ins=[t_xl1_sh.ap().opt()], outs=[t_xl1_full.ap().opt()],
            )
            gat_layer(1)
            nc.gpsimd.collective_compute(
                "AllGather", mybir.AluOpType.bypass, replica_groups=groups,
                ins=[t_xlfs_sh.ap().opt()], outs=[t_xlfs_full.ap().opt()],
            )
            gat_layer(2)
            nc.sync.dma_start(t_xout[:, :], xout_sb[:])

    nc.compile()
    return nc


# ----------------------------------------------------------------------------
# Entry point
# ----------------------------------------------------------------------------

_LAST_RESULTS = None
_LAST_BUILD = None


def bench_last(iters=5):
    """Steady-state execution timing of the last-built kernel.

    Jits the bass custom-call once, device_puts the sharded inputs once,
    then times `iters` repeated executions (block_until_ready).  Returns
    min wall seconds per execution.
    """
    import time
    import jax
    from jax.sharding import Mesh, PartitionSpec, NamedSharding
    from jax.experimental.shard_map import shard_map
    from concourse import bass2jax, mybir as mb

    nc, in_maps, n_cores = _LAST_BUILD
    bass2jax.install_neuronx_cc_hook()

    partition_name = nc.partition_id_tensor.name if nc.partition_id_tensor else None
    in_names, out_names, out_avals, zero_outs = [], [], [], []
    for alloc in nc.m.functions[0].allocations:
        if not isinstance(alloc, mb.MemoryLocationSet):
            continue
        name = alloc.memorylocations[0].name
        if alloc.kind == "ExternalInput":
            if name != partition_name:
                in_names.append(name)
        elif alloc.kind == "ExternalOutput":
            shape = tuple(alloc.tensor_shape)
            dtype = mb.dt.np(alloc.dtype)
            out_names.append(name)
            out_avals.append(jax.core.ShapedArray(shape, dtype))
            zero_outs.append(np.zeros(shape, dtype))
    n_params = len(in_names)
    in_names_all = in_names + out_names
    if partition_name is not None:
        in_names_all.append(partition_name)

    def _body(*args):
        operands = list(args)
        if partition_name is not None:
            operands.append(bass2jax.partition_id_tensor())
        outs = bass2jax._bass_exec_p.bind(
            *operands,
            out_avals=tuple(out_avals),
            in_names=tuple(in_names_all),
            out_names=tuple(out_names),
            lowering_input_output_aliases=(),
            sim_require_finite=True,
            sim_require_nnan=True,
            nc=nc,
        )
        return tuple(outs)

    devices = jax.devices()[:n_cores]
    mesh = Mesh(np.asarray(devices), ("core",))
    spec = PartitionSpec("core")
    sharded = jax.jit(
        shard_map(
            _body, mesh=mesh,
            in_specs=(spec,) * (n_params + len(out_names)),
            out_specs=(spec,) * len(out_names),
            check_rep=False,
        ),
        keep_unused=True,
    )
    sh = NamedSharding(mesh, spec)
    concat_in = [
        jax.device_put(
            np.concatenate([np.asarray(in_maps[c][nm]) for c in range(n_cores)], axis=0), sh
        )
        for nm in in_names
    ]
    concat_zeros = [
        jax.device_put(np.zeros((n_cores * z.shape[0], *z.shape[1:]), z.dtype), sh)
        for z in zero_outs
    ]
    # warmup (compiles)
    out = sharded(*concat_in, *concat_zeros)
    jax.block_until_ready(out)
    times = []
    for _ in range(iters):
        t0 = time.perf_counter()
        out = sharded(*concat_in, *concat_zeros)
        jax.block_until_ready(out)
        times.append(time.perf_counter() - t0)
    return min(times)


def kernel(x, edge_index, edge_attr, batch, mask, params, _trace=False, _cfg=None):
    global _LAST_RESULTS
    from concourse.bass_utils import run_bass_kernel_spmd

    cfg = _cfg or Cfg()
    in_maps, meta = prep_host(x, edge_index, edge_attr, batch, mask, cfg)
    pdata, scalars = pack_params(params, cfg)
    for m in in_maps:
        m.update(pdata)
    pshapes = {k: v.shape for k, v in pdata.items()}

    global _LAST_BUILD
    nc = build_kernel(meta, pshapes, scalars, cfg)
    _LAST_BUILD = (nc, in_maps, cfg.NC)
    res = run_bass_kernel_spmd(
        nc, in_maps, core_ids=list(range(cfg.NC)), trace=False
    )
    _LAST_RESULTS = res

    x_out = np.empty(cfg.N, dtype=np.float32)
    gtot = np.zeros(cfg.B, dtype=np.float32)
    for c in range(cfg.NC):
        r = res.results[c]
        sh = np.asarray(r["x_out_sh"]).reshape(P, cfg.NT)
        x_out[c * cfg.NSH : (c + 1) * cfg.NSH] = sh.T.reshape(-1)[: cfg.NSH]
        gtot += np.asarray(r["gsum"]).reshape(-1)

    counts = np.maximum(
        np.bincount(np.asarray(batch, dtype=np.int64), minlength=cfg.B).astype(
            np.float32
        ),
        1.0,
    )
    x_out_sat = gtot / counts
    return x_out, x_out_sat
